# revision 14
# baseline (speedup 1.0000x reference)
# Trainium2 Bass kernel for the data-reuploading quantum-circuit model
# (nn_DARUAN_45311904972849).
#
# Math: per (batch, dim) element the complex 2-state evolves through
# 8 reps of RZ(t0)·RY(t1)·RZ(w·x+b) plus a final RZ·RY, then <Z>.
# Tracked as a real Bloch vector (nx, ny, nz) instead:
#   RZ(t): (nx, ny) <- (nx cos t - ny sin t, nx sin t + ny cos t)
#   RY(t): (nx, nz) <- (nx cos t + nz sin t, -nx sin t + nz cos t)
# All per-dim RZ angles fold into the data-dependent angle biases on the
# host, so the device loop per rep is: RY (per-partition scalars) then
# RZ by e = w*x + b~ (per-element trig via ACT Sin after a one-period
# range wrap on the DVE). Output z = nz -> affine postact, folded into
# per-partition readout scalars.
#
# Sharding: dim axis split across the 8 cores (256 dims each); each core
# sees the full batch. x is transposed on the host so SBUF tiles are
# (128 dims x batch-chunk) with dims on partitions and all per-(dim,rep)
# parameters as per-partition scalars.
import sys

sys.path.insert(0, '/opt/trn_rl_repo')
from contextlib import ExitStack

import numpy as np

import concourse.bass as bass  # noqa: F401  (bass types used via bacc/tile)
import concourse.tile as tile
from concourse import bacc, mybir
from concourse.bass_utils import run_bass_kernel_spmd

AFT = mybir.ActivationFunctionType
ALU = mybir.AluOpType
F32 = mybir.dt.float32
F16 = mybir.dt.float16

BATCH, DIM, REPS = 4096, 2048, 8
NCORES = 8
DPC = DIM // NCORES          # dims per core
PTILES = DPC // 128          # d-tiles per core
FCH = 2048                   # batch chunk (free dim)
BCH = BATCH // FCH
PI = float(np.pi)

# param column layout (per dim)
_W = 0            # w_r               cols 0..7
_BS = 8           # folded sin bias   cols 8..15
_CT = 16          # cos(t1_r), r=1..7 cols 16..22  (index r-1)
_ST = 23          # sin(t1_r), r=1..7 cols 23..29
_NST = 30         # -sin(t1_r)        cols 30..36
_NXP, _NYP, _NYN, _K1, _K2, _AX, _AZ, _PB, _PI2 = 37, 38, 39, 40, 41, 42, 43, 44, 45
NPARAM = 48

_CACHE = {}


def _build():
    nc = bacc.Bacc('TRN2', target_bir_lowering=False, debug=False,
                   num_devices=NCORES)
    xt_ext = nc.declare_dram_parameter("xt", [DPC, BATCH], F32, isOutput=False)
    pp_ext = nc.declare_dram_parameter("pp", [DPC, NPARAM], F32, isOutput=False)
    yt_ext = nc.declare_dram_parameter("yt", [DPC, BATCH], F32, isOutput=True)

    with ExitStack() as ctx:
        tc = ctx.enter_context(tile.TileContext(nc))
        ppool = ctx.enter_context(tc.tile_pool(name="pp", bufs=2))
        xpool = ctx.enter_context(tc.tile_pool(name="xp", bufs=2))
        apool = ctx.enter_context(tc.tile_pool(name="ang", bufs=2))
        tpool = ctx.enter_context(tc.tile_pool(name="trig", bufs=2))
        spool = ctx.enter_context(tc.tile_pool(name="state", bufs=2))
        mpool = ctx.enter_context(tc.tile_pool(name="tmp", bufs=3))
        opool = ctx.enter_context(tc.tile_pool(name="out", bufs=2))

        for dt in range(PTILES):
            pt = ppool.tile([128, NPARAM], F32, tag="pt")
            nc.sync.dma_start(pt[:], pp_ext[dt * 128:(dt + 1) * 128, :])

            def col(i):
                return pt[:, i:i + 1]

            for bc in range(BCH):
                xt = xpool.tile([128, FCH], F32, tag="x")
                nc.sync.dma_start(
                    xt[:], xt_ext[dt * 128:(dt + 1) * 128,
                                  bc * FCH:(bc + 1) * FCH])

                X = Y = Z = None
                for r in range(REPS):
                    # angle v = w_r*x + bs_r on ACT, then one-period wraps
                    V = apool.tile([128, FCH], F32, tag="V")
                    nc.scalar.activation(V[:], xt[:], AFT.Identity,
                                         bias=col(_BS + r), scale=col(_W + r))
                    US = apool.tile([128, FCH], F32, tag="US")
                    nc.vector.add_range_wrap(US[:], V[:], 0.0, PI, 2 * PI)
                    UA = apool.tile([128, FCH], F32, tag="UA")
                    nc.scalar.activation(UA[:], US[:], AFT.Abs, bias=0.0, scale=1.0)
                    S = tpool.tile([128, FCH], F16, tag="S")
                    nc.scalar.activation(S[:], US[:], AFT.Sin, bias=0.0, scale=1.0)
                    # cos(e) = cos(us) = sin(pi/2 - |us|), arg stays in [-pi/2, pi/2]
                    C = tpool.tile([128, FCH], F16, tag="C")
                    nc.scalar.activation(C[:], UA[:], AFT.Sin, bias=col(_PI2), scale=-1.0)

                    if r == 0:
                        # state after host-folded RZ(t0_0)·RY(t1_0) is the
                        # per-dim scalar vector (nxp, nyp, nzp); apply RZ(e_0).
                        T0 = mpool.tile([128, FCH], F16, tag="M1")
                        nc.vector.tensor_scalar_mul(T0[:], C[:], col(_NXP))
                        T2 = mpool.tile([128, FCH], F16, tag="M3")
                        nc.vector.tensor_scalar_mul(T2[:], S[:], col(_NYN))
                        Xn = spool.tile([128, FCH], F16, tag="X")
                        nc.vector.tensor_add(Xn[:], T0[:], T2[:])
                        T1 = mpool.tile([128, FCH], F16, tag="M2")
                        nc.vector.tensor_scalar_mul(T1[:], C[:], col(_NYP))
                        T3 = mpool.tile([128, FCH], F16, tag="M4")
                        nc.vector.tensor_scalar_mul(T3[:], S[:], col(_NXP))
                        Yn = spool.tile([128, FCH], F16, tag="Y")
                        nc.vector.tensor_add(Yn[:], T1[:], T3[:])
                        X, Y, Z = Xn, Yn, None
                        continue

                    # RY(t1_r)
                    if r == 1:
                        # nz is still the per-dim scalar nzp
                        U = mpool.tile([128, FCH], F16, tag="U")
                        nc.vector.tensor_scalar(
                            U[:], X[:], col(_CT), col(_K1), ALU.mult, ALU.add)
                        Zn = spool.tile([128, FCH], F16, tag="Z")
                        nc.vector.tensor_scalar(
                            Zn[:], X[:], col(_NST), col(_K2), ALU.mult, ALU.add)
                    else:
                        A = mpool.tile([128, FCH], F16, tag="M1")
                        nc.vector.tensor_scalar_mul(A[:], X[:], col(_CT + r - 1))
                        A2 = mpool.tile([128, FCH], F16, tag="M2")
                        nc.vector.tensor_scalar_mul(A2[:], Z[:], col(_ST + r - 1))
                        U = mpool.tile([128, FCH], F16, tag="U")
                        nc.vector.tensor_add(U[:], A[:], A2[:])
                        B = mpool.tile([128, FCH], F16, tag="M3")
                        nc.scalar.mul(B[:], X[:], col(_NST + r - 1))
                        B2 = mpool.tile([128, FCH], F16, tag="M4")
                        nc.scalar.mul(B2[:], Z[:], col(_CT + r - 1))
                        Zn = spool.tile([128, FCH], F16, tag="Z")
                        if True:
                            nc.gpsimd.tensor_add(Zn[:], B[:], B2[:])
                        else:
                            nc.vector.tensor_add(Zn[:], B[:], B2[:])

                    # RZ(e_r): (U, Y) -> (X', Y') rotated by per-element (C, S)
                    M1 = mpool.tile([128, FCH], F16, tag="M1")
                    nc.vector.tensor_mul(M1[:], C[:], U[:])
                    M2 = mpool.tile([128, FCH], F16, tag="M2")
                    nc.gpsimd.tensor_mul(M2[:], S[:], Y[:])
                    Xn = spool.tile([128, FCH], F16, tag="X")
                    nc.vector.tensor_sub(Xn[:], M1[:], M2[:])
                    M3 = mpool.tile([128, FCH], F16, tag="M3")
                    nc.vector.tensor_mul(M3[:], S[:], U[:])
                    M4 = mpool.tile([128, FCH], F16, tag="M4")
                    nc.gpsimd.tensor_mul(M4[:], C[:], Y[:])
                    Yn = spool.tile([128, FCH], F16, tag="Y")
                    if r in (2, 4, 6):
                        nc.gpsimd.tensor_add(Yn[:], M3[:], M4[:])
                    else:
                        nc.vector.tensor_add(Yn[:], M3[:], M4[:])
                    X, Y, Z = Xn, Yn, Zn

                # readout: out = Ax*nx + Az*nz + pb
                O1 = opool.tile([128, FCH], F32, tag="O1")
                nc.vector.tensor_scalar(
                    O1[:], X[:], col(_AX), col(_PB), ALU.mult, ALU.add)
                O = opool.tile([128, FCH], F32, tag="O")
                nc.vector.scalar_tensor_tensor(
                    O[:], Z[:], col(_AZ), O1[:], ALU.mult, ALU.add)
                nc.sync.dma_start(
                    yt_ext[dt * 128:(dt + 1) * 128, bc * FCH:(bc + 1) * FCH],
                    O[:])

    nc.compile()
    return nc


def _fold_params(theta, pw, pb_, ow, ob):
    th = np.asarray(theta, np.float64)
    pw = np.asarray(pw, np.float64)
    pb_ = np.asarray(pb_, np.float64)
    ow = np.asarray(ow, np.float64)
    ob = np.asarray(ob, np.float64)
    t0 = th[:, :REPS, 0]
    t1 = th[:, :REPS, 1]
    tf0 = th[:, REPS, 0]
    tf1 = th[:, REPS, 1]

    P = np.zeros((DIM, NPARAM), np.float64)
    P[:, _W:_W + REPS] = pw
    bs = pb_.copy()
    bs[:, :REPS - 1] += t0[:, 1:]
    bs[:, REPS - 1] += tf0
    P[:, _BS:_BS + REPS] = bs
    ct = np.cos(t1)
    st = np.sin(t1)
    P[:, _CT:_CT + 7] = ct[:, 1:]
    P[:, _ST:_ST + 7] = st[:, 1:]
    P[:, _NST:_NST + 7] = -st[:, 1:]
    nxp = ct[:, 0] * np.cos(t0[:, 0])
    nyp = np.sin(t0[:, 0])
    nzp = -st[:, 0] * np.cos(t0[:, 0])
    P[:, _NXP] = nxp
    P[:, _NYP] = nyp
    P[:, _NYN] = -nyp
    P[:, _K1] = st[:, 1] * nzp
    P[:, _K2] = ct[:, 1] * nzp
    P[:, _AX] = -ow * np.sin(tf1)
    P[:, _AZ] = ow * np.cos(tf1)
    P[:, _PB] = ob
    P[:, _PI2] = np.pi / 2
    return P.astype(np.float32)


def _prep_in_maps(x, theta, preacts_weight, preacts_bias, postact_weights,
                  postact_bias):
    x = np.asarray(x, np.float32)
    P = _fold_params(theta, preacts_weight, preacts_bias, postact_weights,
                     postact_bias)
    in_maps = []
    for c in range(NCORES):
        sl = slice(c * DPC, (c + 1) * DPC)
        in_maps.append({
            "xt": np.ascontiguousarray(x[:, sl].T),
            "pp": np.ascontiguousarray(P[sl]),
        })
    return in_maps


def _gather(results):
    out = np.empty((BATCH, DIM), np.float32)
    for c, r in enumerate(results):
        out[:, c * DPC:(c + 1) * DPC] = r["yt"].T
    return out


def kernel(x, theta, preacts_weight, preacts_bias, postact_weights,
           postact_bias):
    if "nc" not in _CACHE:
        _CACHE["nc"] = _build()
    nc = _CACHE["nc"]
    in_maps = _prep_in_maps(x, theta, preacts_weight, preacts_bias,
                            postact_weights, postact_bias)
    try:
        res = run_bass_kernel_spmd(nc, in_maps, list(range(NCORES)))
    except Exception:
        # transient device errors (e.g. a wedged core from a prior run)
        # usually clear on retry
        res = run_bass_kernel_spmd(nc, in_maps, list(range(NCORES)))
    return _gather(res.results)


def run_traced(inputs, trace_cores=None):
    """test harness helper: returns (out, exec_time_ns)."""
    if "nc" not in _CACHE:
        _CACHE["nc"] = _build()
    nc = _CACHE["nc"]
    in_maps = _prep_in_maps(**inputs)
    res = run_bass_kernel_spmd(nc, in_maps, list(range(NCORES)), trace=True,
                               trace_cores=trace_cores)
    return _gather(res.results), res.exec_time_ns


# revision 18
# speedup vs baseline: 1.1381x; 1.1381x over previous
# Trainium2 Bass kernel for the data-reuploading quantum-circuit model
# (nn_DARUAN_45311904972849).
#
# Math: per (batch, dim) element the complex 2-state evolves through
# 8 reps of RZ(t0)·RY(t1)·RZ(w·x+b) plus a final RZ·RY, then <Z>.
# Tracked as a real Bloch vector (nx, ny, nz) instead:
#   RZ(t): (nx, ny) <- (nx cos t - ny sin t, nx sin t + ny cos t)
#   RY(t): (nx, nz) <- (nx cos t + nz sin t, -nx sin t + nz cos t)
# All per-dim RZ angles fold into the data-dependent angle biases on the
# host, so the device loop per rep is: RY (per-partition scalars) then
# RZ by e = w*x + b~ (per-element trig via ACT Sin after a one-period
# range wrap on the DVE). Output z = nz -> affine postact, folded into
# per-partition readout scalars.
#
# Sharding: dim axis split across the 8 cores (256 dims each); each core
# sees the full batch. x is transposed on the host so SBUF tiles are
# (128 dims x batch-chunk) with dims on partitions and all per-(dim,rep)
# parameters as per-partition scalars.
import sys

sys.path.insert(0, '/opt/trn_rl_repo')
from contextlib import ExitStack

import numpy as np

import concourse.bass as bass  # noqa: F401  (bass types used via bacc/tile)
import concourse.tile as tile
from concourse import bacc, mybir
from concourse.bass_utils import run_bass_kernel_spmd

AFT = mybir.ActivationFunctionType
ALU = mybir.AluOpType
F32 = mybir.dt.float32
F16 = mybir.dt.float16

# ---- fused affine + one-period range wrap as a custom DVE op -------------
# out = wrap(x*w + b) into [-pi, pi], one period. The bound compare uses
# the doubled value (2y vs 2pi) so only the period constant is needed:
# exactly 8 ALU stages.
from concourse.dve_spec import Spec, Src0, C0, C1, C2, Zero  # noqa: E402
from concourse.dve_ops import DveOp, OPS  # noqa: E402


def _wrap_affine_ref(in0, in1, s0, s1, imm2):
    y = in0 * s0 + s1
    d = y + y
    return y + imm2 * ((d < -imm2).astype(np.float32)
                       - (d > imm2).astype(np.float32))


def _register_wrap_affine():
    for op in OPS:
        if op.name == "WRAP_AFFINE_DARUAN":
            return op
    _y = Src0 * C0 + C1
    _d = _y + _y
    spec = Spec(body=_y + C2 * ((_d < (Zero - C2)) - (_d > C2)),
                reference=_wrap_affine_ref)
    op = DveOp("WRAP_AFFINE_DARUAN", spec, subdim=False,
               uops_sha={"v3": "", "v4": ""})
    OPS.append(op)
    import concourse.dve_ops as _dops
    _dops.CUSTOM_DVE_SPECS[op.name] = op.spec
    _dops._SUB_OPCODE_FOR_NAME[op.name] = (
        _dops._CUSTOM_DVE_ROW_BASE + len(OPS) - 1)
    assert _dops._SUB_OPCODE_FOR_NAME[op.name] < 0x20
    import re as _re
    for ver in ("v3", "v4"):
        try:
            op.compile(ver)
        except ValueError as e:
            m = _re.search(r'="([0-9a-f]{16})"', str(e))
            if not m:
                raise
            op.uops_sha[ver] = m.group(1)
            op.compile(ver)
    return op


WRAP_AFFINE = _register_wrap_affine()

BATCH, DIM, REPS = 4096, 2048, 8
NCORES = 8
DPC = DIM // NCORES          # dims per core
PTILES = DPC // 128          # d-tiles per core
FCH = 2048                   # batch chunk (free dim)
BCH = BATCH // FCH
PI = float(np.pi)

# param column layout (per dim)
_W = 0            # w_r               cols 0..7
_BS = 8           # folded sin bias   cols 8..15
_CT = 16          # cos(t1_r), r=1..7 cols 16..22  (index r-1)
_ST = 23          # sin(t1_r), r=1..7 cols 23..29
_NST = 30         # -sin(t1_r)        cols 30..36
_NXP, _NYP, _NYN, _K1, _K2, _AX, _AZ, _PB, _PI2 = 37, 38, 39, 40, 41, 42, 43, 44, 45
NPARAM = 48

_CACHE = {}


def _build():
    nc = bacc.Bacc('TRN2', target_bir_lowering=False, debug=False,
                   num_devices=NCORES)
    xt_ext = nc.declare_dram_parameter("xt", [DPC, BATCH], F32, isOutput=False)
    pp_ext = nc.declare_dram_parameter("pp", [DPC, NPARAM], F32, isOutput=False)
    yt_ext = nc.declare_dram_parameter("yt", [DPC, BATCH], F32, isOutput=True)

    with ExitStack() as ctx:
        tc = ctx.enter_context(tile.TileContext(nc))
        ppool = ctx.enter_context(tc.tile_pool(name="pp", bufs=2))
        xpool = ctx.enter_context(tc.tile_pool(name="xp", bufs=2))
        apool = ctx.enter_context(tc.tile_pool(name="ang", bufs=2))
        tpool = ctx.enter_context(tc.tile_pool(name="trig", bufs=2))
        spool = ctx.enter_context(tc.tile_pool(name="state", bufs=2))
        mpool = ctx.enter_context(tc.tile_pool(name="tmp", bufs=3))
        opool = ctx.enter_context(tc.tile_pool(name="out", bufs=2))

        for dt in range(PTILES):
            pt = ppool.tile([128, NPARAM], F32, tag="pt")
            nc.sync.dma_start(pt[:], pp_ext[dt * 128:(dt + 1) * 128, :])

            def col(i):
                return pt[:, i:i + 1]

            for bc in range(BCH):
                xt = xpool.tile([128, FCH], F32, tag="x")
                nc.sync.dma_start(
                    xt[:], xt_ext[dt * 128:(dt + 1) * 128,
                                  bc * FCH:(bc + 1) * FCH])

                X = Y = Z = None
                for r in range(REPS):
                    # us = wrap(w_r*x + bs_r) in one fused custom DVE op
                    US = apool.tile([128, FCH], F32, tag="US")
                    nc.vector._custom_dve(
                        WRAP_AFFINE, out=US[:], in0=xt[:],
                        s0=col(_W + r), s1=col(_BS + r), imm2=2 * PI)
                    UA = apool.tile([128, FCH], F32, tag="UA")
                    nc.scalar.activation(UA[:], US[:], AFT.Abs, bias=0.0, scale=1.0)
                    S = tpool.tile([128, FCH], F16, tag="S")
                    nc.scalar.activation(S[:], US[:], AFT.Sin, bias=0.0, scale=1.0)
                    # cos(e) = cos(us) = sin(pi/2 - |us|), arg stays in [-pi/2, pi/2]
                    C = tpool.tile([128, FCH], F16, tag="C")
                    nc.scalar.activation(C[:], UA[:], AFT.Sin, bias=col(_PI2), scale=-1.0)

                    if r == 0:
                        # state after host-folded RZ(t0_0)·RY(t1_0) is the
                        # per-dim scalar vector (nxp, nyp, nzp); apply RZ(e_0).
                        T0 = mpool.tile([128, FCH], F16, tag="M1")
                        nc.vector.tensor_scalar_mul(T0[:], C[:], col(_NXP))
                        T2 = mpool.tile([128, FCH], F16, tag="M3")
                        nc.vector.tensor_scalar_mul(T2[:], S[:], col(_NYN))
                        Xn = spool.tile([128, FCH], F16, tag="X")
                        nc.vector.tensor_add(Xn[:], T0[:], T2[:])
                        T1 = mpool.tile([128, FCH], F16, tag="M2")
                        nc.vector.tensor_scalar_mul(T1[:], C[:], col(_NYP))
                        T3 = mpool.tile([128, FCH], F16, tag="M4")
                        nc.vector.tensor_scalar_mul(T3[:], S[:], col(_NXP))
                        Yn = spool.tile([128, FCH], F16, tag="Y")
                        nc.vector.tensor_add(Yn[:], T1[:], T3[:])
                        X, Y, Z = Xn, Yn, None
                        continue

                    # RY(t1_r)
                    if r == 1:
                        # nz is still the per-dim scalar nzp
                        U = mpool.tile([128, FCH], F16, tag="U")
                        nc.vector.tensor_scalar(
                            U[:], X[:], col(_CT), col(_K1), ALU.mult, ALU.add)
                        Zn = spool.tile([128, FCH], F16, tag="Z")
                        nc.vector.tensor_scalar(
                            Zn[:], X[:], col(_NST), col(_K2), ALU.mult, ALU.add)
                    else:
                        A = mpool.tile([128, FCH], F16, tag="M1")
                        nc.vector.tensor_scalar_mul(A[:], X[:], col(_CT + r - 1))
                        A2 = mpool.tile([128, FCH], F16, tag="M2")
                        nc.vector.tensor_scalar_mul(A2[:], Z[:], col(_ST + r - 1))
                        U = mpool.tile([128, FCH], F16, tag="U")
                        nc.vector.tensor_add(U[:], A[:], A2[:])
                        B = mpool.tile([128, FCH], F16, tag="M3")
                        nc.scalar.mul(B[:], X[:], col(_NST + r - 1))
                        B2 = mpool.tile([128, FCH], F16, tag="M4")
                        nc.scalar.mul(B2[:], Z[:], col(_CT + r - 1))
                        Zn = spool.tile([128, FCH], F16, tag="Z")
                        if True:
                            nc.gpsimd.tensor_add(Zn[:], B[:], B2[:])
                        else:
                            nc.vector.tensor_add(Zn[:], B[:], B2[:])

                    # RZ(e_r): (U, Y) -> (X', Y') rotated by per-element (C, S)
                    M1 = mpool.tile([128, FCH], F16, tag="M1")
                    nc.vector.tensor_mul(M1[:], C[:], U[:])
                    M2 = mpool.tile([128, FCH], F16, tag="M2")
                    nc.gpsimd.tensor_mul(M2[:], S[:], Y[:])
                    Xn = spool.tile([128, FCH], F16, tag="X")
                    nc.vector.tensor_sub(Xn[:], M1[:], M2[:])
                    M3 = mpool.tile([128, FCH], F16, tag="M3")
                    nc.vector.tensor_mul(M3[:], S[:], U[:])
                    M4 = mpool.tile([128, FCH], F16, tag="M4")
                    nc.gpsimd.tensor_mul(M4[:], C[:], Y[:])
                    Yn = spool.tile([128, FCH], F16, tag="Y")
                    if r in (2, 6):
                        nc.gpsimd.tensor_add(Yn[:], M3[:], M4[:])
                    else:
                        nc.vector.tensor_add(Yn[:], M3[:], M4[:])
                    X, Y, Z = Xn, Yn, Zn

                # readout: out = Ax*nx + Az*nz + pb
                O1 = opool.tile([128, FCH], F32, tag="O1")
                nc.scalar.activation(O1[:], X[:], AFT.Identity,
                                     bias=col(_PB), scale=col(_AX))
                O = opool.tile([128, FCH], F32, tag="O")
                nc.vector.scalar_tensor_tensor(
                    O[:], Z[:], col(_AZ), O1[:], ALU.mult, ALU.add)
                nc.sync.dma_start(
                    yt_ext[dt * 128:(dt + 1) * 128, bc * FCH:(bc + 1) * FCH],
                    O[:])

    nc.compile()
    return nc


def _fold_params(theta, pw, pb_, ow, ob):
    th = np.asarray(theta, np.float64)
    pw = np.asarray(pw, np.float64)
    pb_ = np.asarray(pb_, np.float64)
    ow = np.asarray(ow, np.float64)
    ob = np.asarray(ob, np.float64)
    t0 = th[:, :REPS, 0]
    t1 = th[:, :REPS, 1]
    tf0 = th[:, REPS, 0]
    tf1 = th[:, REPS, 1]

    P = np.zeros((DIM, NPARAM), np.float64)
    P[:, _W:_W + REPS] = pw
    bs = pb_.copy()
    bs[:, :REPS - 1] += t0[:, 1:]
    bs[:, REPS - 1] += tf0
    P[:, _BS:_BS + REPS] = bs
    ct = np.cos(t1)
    st = np.sin(t1)
    P[:, _CT:_CT + 7] = ct[:, 1:]
    P[:, _ST:_ST + 7] = st[:, 1:]
    P[:, _NST:_NST + 7] = -st[:, 1:]
    nxp = ct[:, 0] * np.cos(t0[:, 0])
    nyp = np.sin(t0[:, 0])
    nzp = -st[:, 0] * np.cos(t0[:, 0])
    P[:, _NXP] = nxp
    P[:, _NYP] = nyp
    P[:, _NYN] = -nyp
    P[:, _K1] = st[:, 1] * nzp
    P[:, _K2] = ct[:, 1] * nzp
    P[:, _AX] = -ow * np.sin(tf1)
    P[:, _AZ] = ow * np.cos(tf1)
    P[:, _PB] = ob
    P[:, _PI2] = np.pi / 2
    return P.astype(np.float32)


def _prep_in_maps(x, theta, preacts_weight, preacts_bias, postact_weights,
                  postact_bias):
    x = np.asarray(x, np.float32)
    P = _fold_params(theta, preacts_weight, preacts_bias, postact_weights,
                     postact_bias)
    in_maps = []
    for c in range(NCORES):
        sl = slice(c * DPC, (c + 1) * DPC)
        in_maps.append({
            "xt": np.ascontiguousarray(x[:, sl].T),
            "pp": np.ascontiguousarray(P[sl]),
        })
    return in_maps


def _gather(results):
    out = np.empty((BATCH, DIM), np.float32)
    for c, r in enumerate(results):
        out[:, c * DPC:(c + 1) * DPC] = r["yt"].T
    return out


def kernel(x, theta, preacts_weight, preacts_bias, postact_weights,
           postact_bias):
    if "nc" not in _CACHE:
        _CACHE["nc"] = _build()
    nc = _CACHE["nc"]
    in_maps = _prep_in_maps(x, theta, preacts_weight, preacts_bias,
                            postact_weights, postact_bias)
    try:
        res = run_bass_kernel_spmd(nc, in_maps, list(range(NCORES)))
    except Exception:
        # transient device errors (e.g. a wedged core from a prior run)
        # usually clear on retry
        res = run_bass_kernel_spmd(nc, in_maps, list(range(NCORES)))
    return _gather(res.results)


def run_traced(inputs, trace_cores=None):
    """test harness helper: returns (out, exec_time_ns)."""
    if "nc" not in _CACHE:
        _CACHE["nc"] = _build()
    nc = _CACHE["nc"]
    in_maps = _prep_in_maps(**inputs)
    res = run_bass_kernel_spmd(nc, in_maps, list(range(NCORES)), trace=True,
                               trace_cores=trace_cores)
    return _gather(res.results), res.exec_time_ns


# revision 19
# speedup vs baseline: 1.1419x; 1.0034x over previous
# Trainium2 Bass kernel for the data-reuploading quantum-circuit model
# (nn_DARUAN_45311904972849).
#
# Math: per (batch, dim) element the complex 2-state evolves through
# 8 reps of RZ(t0)·RY(t1)·RZ(w·x+b) plus a final RZ·RY, then <Z>.
# Tracked as a real Bloch vector (nx, ny, nz) instead:
#   RZ(t): (nx, ny) <- (nx cos t - ny sin t, nx sin t + ny cos t)
#   RY(t): (nx, nz) <- (nx cos t + nz sin t, -nx sin t + nz cos t)
# All per-dim RZ angles fold into the data-dependent angle biases on the
# host, so the device loop per rep is: RY (per-partition scalars) then
# RZ by e = w*x + b~ (per-element trig via ACT Sin after a one-period
# range wrap on the DVE). Output z = nz -> affine postact, folded into
# per-partition readout scalars.
#
# Sharding: dim axis split across the 8 cores (256 dims each); each core
# sees the full batch. x is transposed on the host so SBUF tiles are
# (128 dims x batch-chunk) with dims on partitions and all per-(dim,rep)
# parameters as per-partition scalars.
import sys

sys.path.insert(0, '/opt/trn_rl_repo')
from contextlib import ExitStack

import numpy as np

import concourse.bass as bass  # noqa: F401  (bass types used via bacc/tile)
import concourse.tile as tile
from concourse import bacc, mybir
from concourse.bass_utils import run_bass_kernel_spmd

AFT = mybir.ActivationFunctionType
ALU = mybir.AluOpType
F32 = mybir.dt.float32
F16 = mybir.dt.float16

# ---- fused affine + one-period range wrap as a custom DVE op -------------
# out = wrap(x*w + b) into [-pi, pi], one period. The bound compare uses
# the doubled value (2y vs 2pi) so only the period constant is needed:
# exactly 8 ALU stages.
from concourse.dve_spec import Spec, Src0, C0, C1, C2, Zero  # noqa: E402
from concourse.dve_ops import DveOp, OPS  # noqa: E402


def _wrap_affine_ref(in0, in1, s0, s1, imm2):
    y = in0 * s0 + s1
    d = y + y
    return y + imm2 * ((d < -imm2).astype(np.float32)
                       - (d > imm2).astype(np.float32))


def _register_wrap_affine():
    for op in OPS:
        if op.name == "WRAP_AFFINE_DARUAN":
            return op
    _y = Src0 * C0 + C1
    _d = _y + _y
    spec = Spec(body=_y + C2 * ((_d < (Zero - C2)) - (_d > C2)),
                reference=_wrap_affine_ref)
    op = DveOp("WRAP_AFFINE_DARUAN", spec, subdim=False,
               uops_sha={"v3": "", "v4": ""})
    OPS.append(op)
    import concourse.dve_ops as _dops
    _dops.CUSTOM_DVE_SPECS[op.name] = op.spec
    _dops._SUB_OPCODE_FOR_NAME[op.name] = (
        _dops._CUSTOM_DVE_ROW_BASE + len(OPS) - 1)
    assert _dops._SUB_OPCODE_FOR_NAME[op.name] < 0x20
    import re as _re
    for ver in ("v3", "v4"):
        try:
            op.compile(ver)
        except ValueError as e:
            m = _re.search(r'="([0-9a-f]{16})"', str(e))
            if not m:
                raise
            op.uops_sha[ver] = m.group(1)
            op.compile(ver)
    return op


WRAP_AFFINE = _register_wrap_affine()

BATCH, DIM, REPS = 4096, 2048, 8
NCORES = 8
DPC = DIM // NCORES          # dims per core
PTILES = DPC // 128          # d-tiles per core
FCH = 2048                   # batch chunk (free dim)
BCH = BATCH // FCH
PI = float(np.pi)

# param column layout (per dim)
_W = 0            # w_r               cols 0..7
_BS = 8           # folded sin bias   cols 8..15
_CT = 16          # cos(t1_r), r=1..7 cols 16..22  (index r-1)
_ST = 23          # sin(t1_r), r=1..7 cols 23..29
_NST = 30         # -sin(t1_r)        cols 30..36
_NXP, _NYP, _NYN, _K1, _K2, _AX, _AZ, _PB, _PI2 = 37, 38, 39, 40, 41, 42, 43, 44, 45
NPARAM = 48

_CACHE = {}


def _build():
    nc = bacc.Bacc('TRN2', target_bir_lowering=False, debug=False,
                   num_devices=NCORES)
    xt_ext = nc.declare_dram_parameter("xt", [DPC, BATCH], F32, isOutput=False)
    pp_ext = nc.declare_dram_parameter("pp", [DPC, NPARAM], F32, isOutput=False)
    yt_ext = nc.declare_dram_parameter("yt", [DPC, BATCH], F32, isOutput=True)

    with ExitStack() as ctx:
        tc = ctx.enter_context(tile.TileContext(nc))
        ppool = ctx.enter_context(tc.tile_pool(name="pp", bufs=2))
        xpool = ctx.enter_context(tc.tile_pool(name="xp", bufs=2))
        apool = ctx.enter_context(tc.tile_pool(name="ang", bufs=3))
        tpool = ctx.enter_context(tc.tile_pool(name="trig", bufs=2))
        spool = ctx.enter_context(tc.tile_pool(name="state", bufs=2))
        mpool = ctx.enter_context(tc.tile_pool(name="tmp", bufs=3))
        opool = ctx.enter_context(tc.tile_pool(name="out", bufs=2))

        for dt in range(PTILES):
            pt = ppool.tile([128, NPARAM], F32, tag="pt")
            nc.sync.dma_start(pt[:], pp_ext[dt * 128:(dt + 1) * 128, :])

            def col(i):
                return pt[:, i:i + 1]

            for bc in range(BCH):
                xt = xpool.tile([128, FCH], F32, tag="x")
                nc.sync.dma_start(
                    xt[:], xt_ext[dt * 128:(dt + 1) * 128,
                                  bc * FCH:(bc + 1) * FCH])

                X = Y = Z = None
                for r in range(REPS):
                    # us = wrap(w_r*x + bs_r) in one fused custom DVE op
                    US = apool.tile([128, FCH], F32, tag="US")
                    nc.vector._custom_dve(
                        WRAP_AFFINE, out=US[:], in0=xt[:],
                        s0=col(_W + r), s1=col(_BS + r), imm2=2 * PI)
                    UA = apool.tile([128, FCH], F32, tag="UA")
                    nc.scalar.activation(UA[:], US[:], AFT.Abs, bias=0.0, scale=1.0)
                    S = tpool.tile([128, FCH], F16, tag="S")
                    nc.scalar.activation(S[:], US[:], AFT.Sin, bias=0.0, scale=1.0)
                    # cos(e) = cos(us) = sin(pi/2 - |us|), arg stays in [-pi/2, pi/2]
                    C = tpool.tile([128, FCH], F16, tag="C")
                    nc.scalar.activation(C[:], UA[:], AFT.Sin, bias=col(_PI2), scale=-1.0)

                    if r == 0:
                        # state after host-folded RZ(t0_0)·RY(t1_0) is the
                        # per-dim scalar vector (nxp, nyp, nzp); apply RZ(e_0).
                        T0 = mpool.tile([128, FCH], F16, tag="M1")
                        nc.vector.tensor_scalar_mul(T0[:], C[:], col(_NXP))
                        T2 = mpool.tile([128, FCH], F16, tag="M3")
                        nc.vector.tensor_scalar_mul(T2[:], S[:], col(_NYN))
                        Xn = spool.tile([128, FCH], F16, tag="X")
                        nc.vector.tensor_add(Xn[:], T0[:], T2[:])
                        T1 = mpool.tile([128, FCH], F16, tag="M2")
                        nc.vector.tensor_scalar_mul(T1[:], C[:], col(_NYP))
                        T3 = mpool.tile([128, FCH], F16, tag="M4")
                        nc.vector.tensor_scalar_mul(T3[:], S[:], col(_NXP))
                        Yn = spool.tile([128, FCH], F16, tag="Y")
                        nc.vector.tensor_add(Yn[:], T1[:], T3[:])
                        X, Y, Z = Xn, Yn, None
                        continue

                    # RY(t1_r)
                    if r == 1:
                        # nz is still the per-dim scalar nzp
                        U = mpool.tile([128, FCH], F16, tag="U")
                        nc.vector.tensor_scalar(
                            U[:], X[:], col(_CT), col(_K1), ALU.mult, ALU.add)
                        Zn = spool.tile([128, FCH], F16, tag="Z")
                        nc.scalar.activation(Zn[:], X[:], AFT.Identity,
                                             bias=col(_K2), scale=col(_NST))
                    else:
                        A = mpool.tile([128, FCH], F16, tag="M1")
                        nc.vector.tensor_scalar_mul(A[:], X[:], col(_CT + r - 1))
                        A2 = mpool.tile([128, FCH], F16, tag="M2")
                        nc.vector.tensor_scalar_mul(A2[:], Z[:], col(_ST + r - 1))
                        U = mpool.tile([128, FCH], F16, tag="U")
                        nc.vector.tensor_add(U[:], A[:], A2[:])
                        B = mpool.tile([128, FCH], F16, tag="M3")
                        nc.scalar.mul(B[:], X[:], col(_NST + r - 1))
                        B2 = mpool.tile([128, FCH], F16, tag="M4")
                        nc.scalar.mul(B2[:], Z[:], col(_CT + r - 1))
                        Zn = spool.tile([128, FCH], F16, tag="Z")
                        if True:
                            nc.gpsimd.tensor_add(Zn[:], B[:], B2[:])
                        else:
                            nc.vector.tensor_add(Zn[:], B[:], B2[:])

                    # RZ(e_r): (U, Y) -> (X', Y') rotated by per-element (C, S)
                    M1 = mpool.tile([128, FCH], F16, tag="M1")
                    nc.vector.tensor_mul(M1[:], C[:], U[:])
                    M2 = mpool.tile([128, FCH], F16, tag="M2")
                    nc.gpsimd.tensor_mul(M2[:], S[:], Y[:])
                    Xn = spool.tile([128, FCH], F16, tag="X")
                    nc.vector.tensor_sub(Xn[:], M1[:], M2[:])
                    M3 = mpool.tile([128, FCH], F16, tag="M3")
                    nc.vector.tensor_mul(M3[:], S[:], U[:])
                    M4 = mpool.tile([128, FCH], F16, tag="M4")
                    nc.gpsimd.tensor_mul(M4[:], C[:], Y[:])
                    Yn = spool.tile([128, FCH], F16, tag="Y")
                    if r in (2, 6):
                        nc.gpsimd.tensor_add(Yn[:], M3[:], M4[:])
                    else:
                        nc.vector.tensor_add(Yn[:], M3[:], M4[:])
                    X, Y, Z = Xn, Yn, Zn

                # readout: out = Ax*nx + Az*nz + pb
                O1 = opool.tile([128, FCH], F32, tag="O1")
                nc.scalar.activation(O1[:], X[:], AFT.Identity,
                                     bias=col(_PB), scale=col(_AX))
                O = opool.tile([128, FCH], F32, tag="O")
                nc.vector.scalar_tensor_tensor(
                    O[:], Z[:], col(_AZ), O1[:], ALU.mult, ALU.add)
                nc.sync.dma_start(
                    yt_ext[dt * 128:(dt + 1) * 128, bc * FCH:(bc + 1) * FCH],
                    O[:])

    nc.compile()
    return nc


def _fold_params(theta, pw, pb_, ow, ob):
    th = np.asarray(theta, np.float64)
    pw = np.asarray(pw, np.float64)
    pb_ = np.asarray(pb_, np.float64)
    ow = np.asarray(ow, np.float64)
    ob = np.asarray(ob, np.float64)
    t0 = th[:, :REPS, 0]
    t1 = th[:, :REPS, 1]
    tf0 = th[:, REPS, 0]
    tf1 = th[:, REPS, 1]

    P = np.zeros((DIM, NPARAM), np.float64)
    P[:, _W:_W + REPS] = pw
    bs = pb_.copy()
    bs[:, :REPS - 1] += t0[:, 1:]
    bs[:, REPS - 1] += tf0
    P[:, _BS:_BS + REPS] = bs
    ct = np.cos(t1)
    st = np.sin(t1)
    P[:, _CT:_CT + 7] = ct[:, 1:]
    P[:, _ST:_ST + 7] = st[:, 1:]
    P[:, _NST:_NST + 7] = -st[:, 1:]
    nxp = ct[:, 0] * np.cos(t0[:, 0])
    nyp = np.sin(t0[:, 0])
    nzp = -st[:, 0] * np.cos(t0[:, 0])
    P[:, _NXP] = nxp
    P[:, _NYP] = nyp
    P[:, _NYN] = -nyp
    P[:, _K1] = st[:, 1] * nzp
    P[:, _K2] = ct[:, 1] * nzp
    P[:, _AX] = -ow * np.sin(tf1)
    P[:, _AZ] = ow * np.cos(tf1)
    P[:, _PB] = ob
    P[:, _PI2] = np.pi / 2
    return P.astype(np.float32)


def _prep_in_maps(x, theta, preacts_weight, preacts_bias, postact_weights,
                  postact_bias):
    x = np.asarray(x, np.float32)
    P = _fold_params(theta, preacts_weight, preacts_bias, postact_weights,
                     postact_bias)
    in_maps = []
    for c in range(NCORES):
        sl = slice(c * DPC, (c + 1) * DPC)
        in_maps.append({
            "xt": np.ascontiguousarray(x[:, sl].T),
            "pp": np.ascontiguousarray(P[sl]),
        })
    return in_maps


def _gather(results):
    out = np.empty((BATCH, DIM), np.float32)
    for c, r in enumerate(results):
        out[:, c * DPC:(c + 1) * DPC] = r["yt"].T
    return out


def kernel(x, theta, preacts_weight, preacts_bias, postact_weights,
           postact_bias):
    if "nc" not in _CACHE:
        _CACHE["nc"] = _build()
    nc = _CACHE["nc"]
    in_maps = _prep_in_maps(x, theta, preacts_weight, preacts_bias,
                            postact_weights, postact_bias)
    try:
        res = run_bass_kernel_spmd(nc, in_maps, list(range(NCORES)))
    except Exception:
        # transient device errors (e.g. a wedged core from a prior run)
        # usually clear on retry
        res = run_bass_kernel_spmd(nc, in_maps, list(range(NCORES)))
    return _gather(res.results)


def run_traced(inputs, trace_cores=None):
    """test harness helper: returns (out, exec_time_ns)."""
    if "nc" not in _CACHE:
        _CACHE["nc"] = _build()
    nc = _CACHE["nc"]
    in_maps = _prep_in_maps(**inputs)
    res = run_bass_kernel_spmd(nc, in_maps, list(range(NCORES)), trace=True,
                               trace_cores=trace_cores)
    return _gather(res.results), res.exec_time_ns
